# revision 1
# baseline (speedup 1.0000x reference)
"""AttentionDCA energy kernel for 8 Trainium2 NeuronCores.

Math: with one-hot E_b in {0,1}^{L x 21} for sequence x[b],
    energy[b] = -sum_h <E_b^T A_h E_b, V_h>_F
where A_h = softmax(Q_h K_h^T / d_k).  Everything becomes PE matmuls:

  per core (4 heads, H sharded over 8 cores):
    S_T[j,i]   = K_h Q_h^T                      (scores, transposed layout)
    expS       = exp(S_T / d_k)                 (ScalarE, PSUM->SBUF bf16)
    R'[i,col]  = sum_j expS[j,i] * EA[j,col]    (PE; EA = one-hot for all 64 b
                                                 + a ones column -> row sums r)
    R          = R' * (1/r[i])                  (ScalarE eviction w/ per-partition scale)
    C          = E_g^T R_g                      (PE, 6 b's block-diagonal batched, M=126)
    S[p,g]     = sum(C ⊙ VV)                    (DVE mul+reduce; VV = -V masked)
    energy     = P6^T S                         (PE selector matmul, cross-partition sum)

Host only shards/stages inputs and sums the 8 partial [6,11] outputs.
"""

import numpy as np
import ml_dtypes

# Problem constants (hardcoded per contract)
B, L, H, D, NAA = 64, 512, 32, 128, 21
NCORES = 8
HPC = H // NCORES            # heads per core = 4
JB = L // 128                # 4 position blocks
BG = 6                       # b's per group
NG = (B + BG - 1) // BG      # 11 groups
BPAD = BG * NG               # 66 padded batch
NE = BPAD * NAA              # 1386 one-hot columns
NEA = 1392                   # padded: 3 chunks of <=464
CHK = 464                    # columns per PSUM-bank chunk (chunks 0,1)
ONES_COL = B * NAA           # 1344: column of ones -> softmax row sums
CHK2 = 424                   # chunk 2 streams cols 928..1352 only
RCOL = ONES_COL - 2 * CHK    # 416: ones-column index inside chunk 2
MP = BG * NAA                # 126 used output partitions of mat2
GW = 128                     # group stride in the mat2 E copy (M=128 -> FWL)
NE2 = NG * GW                # 1408 columns of the mat2 E copy
NQ = HPC * MP                # 504 output free size of mat2

_NC = None


def _build_nc():
    import concourse.bacc as bacc
    import concourse.tile as tile
    from concourse import mybir

    f32 = mybir.dt.float32
    bf16 = mybir.dt.bfloat16
    AF = mybir.ActivationFunctionType

    nc = bacc.Bacc("TRN2", target_bir_lowering=False, debug=False)

    qk0_d = nc.dram_tensor("qk0", [128, 2, 2, L], bf16, kind="ExternalInput")
    qk1_d = nc.dram_tensor("qk1", [128, 2, 2, L], bf16, kind="ExternalInput")
    eb_d = nc.dram_tensor("eb", [128, JB, NEA], bf16, kind="ExternalInput")
    e2_d = nc.dram_tensor("e2", [128, JB, NE2], bf16, kind="ExternalInput")
    vv_d = nc.dram_tensor("vv", [128, NQ], f32, kind="ExternalInput")
    p6_d = nc.dram_tensor("p6", [128, BG], f32, kind="ExternalInput")
    out_d = nc.dram_tensor("energy", [BG, NG], f32, kind="ExternalOutput")

    with tile.TileContext(nc) as tc:
        with (
            tc.tile_pool(name="const", bufs=1) as cpool,
            tc.tile_pool(name="exps", bufs=HPC) as xpool,
            tc.tile_pool(name="rall", bufs=1) as rpool,
            tc.tile_pool(name="small", bufs=8) as spool,
            tc.tile_pool(name="psum", bufs=2, space="PSUM") as pp,
        ):
            qk0_sb = cpool.tile([128, 2, 2, L], bf16, tag="qk0")
            qk1_sb = cpool.tile([128, 2, 2, L], bf16, tag="qk1")
            eb_sb = cpool.tile([128, JB, NEA], bf16, tag="eb")
            e2_sb = cpool.tile([128, JB, NE2], bf16, tag="e2")
            vv_sb = cpool.tile([128, NQ], f32, tag="vv")
            p6_sb = cpool.tile([128, BG], f32, tag="p6")
            s_sb = cpool.tile([128, NG], f32, tag="ssb")
            zero_sb = cpool.tile([128, 1], f32, tag="zero")

            # qk gates the first scores; eb (largest, 1.4MB) is only
            # needed once mat1 starts, so it transfers last
            nc.sync.dma_start(out=qk0_sb[:], in_=qk0_d[:])
            nc.sync.dma_start(out=qk1_sb[:], in_=qk1_d[:])
            nc.sync.dma_start(out=vv_sb[:], in_=vv_d[:])
            nc.sync.dma_start(out=p6_sb[:], in_=p6_d[:])
            nc.sync.dma_start(out=eb_sb[:], in_=eb_d[:])
            nc.sync.dma_start(out=e2_sb[:], in_=e2_d[:])
            nc.vector.memset(zero_sb[:], 0.0)

            # Phase 1+2 interleaved: scores+exp for head h are emitted two
            # heads ahead of mat1(h), so PE streams score matmuls for h+2
            # while ScalarE runs exp(h+1) and PE's mat1(h) chews on exp(h).
            exps = []

            def scores(h):
                ex = xpool.tile([128, JB, L], bf16, tag="ex")
                exps.append(ex)
                for jb in range(JB):
                    # borrow the (still idle) big psum slots for half the
                    # first two heads' score tiles so the 4-matmul chain
                    # isn't gated on exp; later heads run beside mat1,
                    # whose pr tiles need those slots
                    ps = pp.tile([128, L], f32,
                                 tag="small" if (jb < 2 or h >= 1) else "big")
                    qq = qk0_sb if h < 2 else qk1_sb
                    nc.tensor.matmul(
                        ps[:],
                        qq[:, 1, h % 2, jb * 128:(jb + 1) * 128],
                        qq[:, 0, h % 2, :],
                        start=True,
                        stop=True,
                    )
                    nc.scalar.activation(
                        ex[:, jb, :], ps[:], AF.Exp,
                        bias=zero_sb[:], scale=1.0 / D,
                    )

            scores(0)
            scores(1)
            r_sb = rpool.tile([128, JB, HPC, NEA], bf16, tag="r")
            # cols [1352:1392) are never written by evictions; zero them so
            # group 10's (discarded pad-b) matmul reads no NaN garbage
            nc.vector.memset(r_sb[:, :, :, 2 * CHK + CHK2:], 0.0)
            for h in range(HPC):
                ex = exps[h]
                for ib in range(JB):
                    pr = pp.tile([128, 3, 512], f32, tag="big")
                    for jb in range(JB):
                        lhs = ex[:, jb, ib * 128:(ib + 1) * 128]
                        # at the last jb, finish chunk 2 first: the ones
                        # column lives there, so the reciprocal overlaps
                        # the remaining chunk matmuls
                        cks = (2, 0, 1) if jb == JB - 1 else (0, 1, 2)
                        for ck in cks:
                            w = CHK2 if ck == 2 else CHK
                            nc.tensor.matmul(
                                pr[:, ck, :w],
                                lhs,
                                eb_sb[:, jb, ck * CHK:ck * CHK + w],
                                start=(jb == 0),
                                stop=(jb == JB - 1),
                            )
                    rcp = spool.tile([128, 1], f32, tag="rcp")
                    nc.vector.reciprocal(rcp[:], pr[:, 2, RCOL:RCOL + 1])
                    # evict with 1/r scale (PSUM f32 -> SBUF bf16), split
                    # across ScalarE (chunks 0-1) and VectorE (chunk 2) so
                    # the psum slot frees faster
                    nc.scalar.mul(
                        r_sb[:, ib, h, :2 * CHK].rearrange("p (k c) -> p k c", k=2),
                        pr[:, 0:2, :CHK],
                        rcp[:],
                    )
                    nc.vector.tensor_scalar_mul(
                        r_sb[:, ib, h, 2 * CHK:2 * CHK + CHK2],
                        pr[:, 2, :CHK2],
                        rcp[:],
                    )
                if h + 2 < HPC:
                    scores(h + 2)

            # Phase 3: C = E^T R (block-diagonal over 6 b's), V-weighted reduce
            for g in range(NG):
                pc = pp.tile([128, NQ], f32, tag="big")
                for ib in range(JB):
                    nc.tensor.matmul(
                        pc[:],
                        e2_sb[:, ib, g * GW:(g + 1) * GW],
                        r_sb[:, ib, :, g * MP:(g + 1) * MP],
                        start=(ib == 0),
                        stop=(ib == JB - 1),
                    )
                scr = spool.tile([128, NQ], f32, tag="scr")
                scr2 = spool.tile([128, NQ], f32, tag="scr2")
                nc.vector.tensor_mul(scr[:], pc[:], vv_sb[:])
                # free-dim sum on ScalarE (accum_out), keeping DVE light
                nc.scalar.activation(
                    scr2[:], scr[:], AF.Copy,
                    accum_out=s_sb[:, g:g + 1],
                )

            # Phase 4: cross-partition sum via selector matmul
            pe = pp.tile([BG, NG], f32, tag="small")
            nc.tensor.matmul(pe[:], p6_sb[:], s_sb[:], start=True, stop=True)
            eout = spool.tile([BG, NG], f32, tag="eout")
            nc.scalar.copy(eout[:], pe[:])
            nc.sync.dma_start(out=out_d[:], in_=eout[:])

    nc.compile()
    return nc


def _get_nc():
    global _NC
    if _NC is None:
        _NC = _build_nc()
    return _NC


def _stage_inputs(x, Q, K, V):
    """Host-side sharding/staging. Returns in_maps for the 8 cores."""
    bf16 = ml_dtypes.bfloat16
    x = np.asarray(x)
    Q = np.asarray(Q, dtype=np.float32)
    K = np.asarray(K, dtype=np.float32)
    V = np.asarray(V, dtype=np.float32)

    # One-hot EA [L, NEA] (+ ones column), replicated to all cores
    onehot = (x[:, :, None] == np.arange(NAA, dtype=x.dtype)[None, None, :])
    ea = np.zeros((L, NEA), dtype=np.float32)
    ea[:, : B * NAA] = onehot.transpose(1, 0, 2).reshape(L, B * NAA)
    ea[:, ONES_COL] = 1.0  # col 1344
    eb_host = np.ascontiguousarray(
        ea.reshape(JB, 128, NEA).transpose(1, 0, 2)
    ).astype(bf16)

    ea2 = np.zeros((L, NE2), dtype=np.float32)
    for g in range(NG):
        nb = min(BG, B - g * BG)
        blk = onehot[g * BG: g * BG + nb].transpose(1, 0, 2).reshape(L, nb * NAA)
        ea2[:, g * GW: g * GW + nb * NAA] = blk
    e2_host = np.ascontiguousarray(
        ea2.reshape(JB, 128, NE2).transpose(1, 0, 2)
    ).astype(bf16)

    p6 = np.zeros((128, BG), dtype=np.float32)
    for bl in range(BG):
        p6[bl * NAA:(bl + 1) * NAA, bl] = 1.0

    in_maps = []
    for c in range(NCORES):
        hs = slice(c * HPC, (c + 1) * HPC)
        qt = Q[hs].transpose(2, 0, 1)
        kt = K[hs].transpose(2, 0, 1)
        qk0 = np.ascontiguousarray(
            np.stack([qt[:, 0:2], kt[:, 0:2]], axis=1)).astype(bf16)
        qk1 = np.ascontiguousarray(
            np.stack([qt[:, 2:4], kt[:, 2:4]], axis=1)).astype(bf16)
        vv = np.zeros((128, NQ), dtype=np.float32)
        vc = V[hs]
        for h in range(HPC):
            for bl in range(BG):
                vv[bl * NAA:(bl + 1) * NAA,
                   h * MP + bl * NAA: h * MP + (bl + 1) * NAA] = -vc[h]
        in_maps.append({"qk0": qk0, "qk1": qk1, "eb": eb_host, "e2": e2_host,
                        "vv": vv, "p6": p6})
    return in_maps


def _run(x, Q, K, V, trace=False):
    from concourse.bass_utils import run_bass_kernel_spmd

    nc = _get_nc()
    in_maps = _stage_inputs(x, Q, K, V)
    res = run_bass_kernel_spmd(nc, in_maps, list(range(NCORES)), trace=trace)

    total = np.zeros((BG, NG), dtype=np.float64)
    for r in res.results:
        total += r["energy"].astype(np.float64)
    bidx = np.arange(B)
    energy = total[bidx % BG, bidx // BG].astype(np.float32)
    return energy, res


def kernel(x, Q, K, V):
    return _run(x, Q, K, V)[0]

